# revision 6
# baseline (speedup 1.0000x reference)
"""Trainium2 Bass kernel for nn_Attention_53712861003822.

RoPE attention block (GQA 32 q-heads / 8 kv-heads, full non-causal softmax)
with fused output projection, tensor-parallel over heads across 8 NeuronCores.

Sharding (per core c):
  - Wq rows [512c, 512c+512)   -> 4 q heads per core (pre-transposed, bf16)
  - Wk/Wv rows [128c, 128c+128) -> 1 kv head per core (GQA group == core)
  - full hidden_states, pre-transposed to [D, B*S] (bf16) on every core
  - attn.T [512, B*S] is AllGathered across cores (bf16, per-batch chunks)
  - Wo rows [512c, 512c+512) transposed -> each core emits output columns
    [512c, 512c+512); host concatenates.

Softmax is computed without max-subtraction (scores are O(1e-3) here;
exp is exact), via the D = exp(s)-1 decomposition so that the tiny
softmax signal survives bf16 matmuls:
  attn.T = (sum_k V[k,:] + D.T @ V) / (S + sum_k D)   computed per head.
"""
import json
import math

import numpy as np
import ml_dtypes

import concourse.bass as bass
import concourse.tile as tile
import concourse.mybir as mybir

BF = mybir.dt.bfloat16
F32 = mybir.dt.float32

CFG_FULL = dict(n_cores=8, B=4, S=1024, D=4096, HD=128, H_LOC=4, PANEL=512, N_QUART=4)
CFG_SMALL = dict(n_cores=8, B=1, S=256, D=512, HD=128, H_LOC=4, PANEL=128, N_QUART=2)


# ---------------------------------------------------------------------------
# BIR post-pass: this walrus build rejects instructions with more than one
# sync wait.  Move extra waits onto fresh single-wait NoOps inserted just
# before the instruction on the same engine stream (engines run a block in
# order, so the conjunction of waits is preserved; a wait's producer is
# always scheduled earlier, so hoisting the wait to issue time is safe).
# ---------------------------------------------------------------------------
def _fix_bir_waits(bir_bytes: bytes, max_waits: int = 1) -> bytes:
    bir = json.loads(bir_bytes)
    n = [0]

    def split(insts):
        out = []
        for inst in insts:
            si = inst.get("sync_info")
            waits = si.get("on_wait") if si else None
            if waits and len(waits) > max_waits:
                for w in waits[:-max_waits]:
                    n[0] += 1
                    out.append({
                        "debug": inst.get("debug", 0),
                        "engine": inst["engine"],
                        "ins": [],
                        "name": f"I-waitsplit-{n[0]}",
                        "opcode": "NoOp",
                        "outs": [],
                        "sync_info": {"on_update": [], "on_wait": [w]},
                    })
                si["on_wait"] = waits[-max_waits:]
            out.append(inst)
        return out

    for func in bir["functions"]:
        for blk in func["blocks"]:
            blk["instructions"] = split(blk["instructions"])
    return json.dumps(bir).encode()


def build_nc(cfg):
    n_cores = cfg["n_cores"]
    B, S, D, HD = cfg["B"], cfg["S"], cfg["D"], cfg["HD"]
    H_LOC, PANEL, N_QUART = cfg["H_LOC"], cfg["PANEL"], cfg["N_QUART"]
    T = B * S
    D_CH = D // 128
    O_LOC = H_LOC * HD
    O_FULL = n_cores * O_LOC
    O_CH = O_FULL // 128
    OUT_SLICE = D // n_cores
    S_CH = S // 128
    P_PER_B = S // PANEL
    HCH = D_CH // 2
    HALF = HD // 2
    SCALE = 1.0 / math.sqrt(HD)
    Exp = mybir.ActivationFunctionType.Exp

    nc = bass.Bass("TRN2", target_bir_lowering=False, debug=False,
                   num_devices=n_cores)

    hsT = nc.dram_tensor("hsT", [D, T], BF, kind="ExternalInput").ap()
    wq = nc.dram_tensor("wq_t", [D, O_LOC], BF, kind="ExternalInput").ap()
    wk = nc.dram_tensor("wk_t", [D, HD], BF, kind="ExternalInput").ap()
    wv = nc.dram_tensor("wv_t", [D, HD], BF, kind="ExternalInput").ap()
    wo = nc.dram_tensor("wo_t", [O_FULL, OUT_SLICE], BF, kind="ExternalInput").ap()
    # cos duplicated on both halves; sin with -/+ sign folded per half
    cos = nc.dram_tensor("cos_t", [HD, S], BF, kind="ExternalInput").ap()
    sin = nc.dram_tensor("sin_t", [HD, S], BF, kind="ExternalInput").ap()
    out = nc.dram_tensor("out", [T, OUT_SLICE], F32, kind="ExternalOutput").ap()

    with tile.TileContext(nc) as tc:
        with (
            tc.tile_pool(name="pw", bufs=1) as pw,
            tc.tile_pool(name="phst", bufs=3) as phst,
            tc.tile_pool(name="pqkv", bufs=2) as pqkv,
            tc.tile_pool(name="praw", bufs=2) as praw,
            tc.tile_pool(name="prt", bufs=2) as prt,
            tc.tile_pool(name="pe", bufs=2) as pe_pool,
            tc.tile_pool(name="pd", bufs=3) as pd,
            tc.tile_pool(name="psmall", bufs=2) as psmall,
            tc.tile_pool(name="pattn", bufs=1) as pattn,
            tc.tile_pool(name="pat", bufs=2) as pat,
            tc.tile_pool(name="pout", bufs=2) as pout,
            tc.tile_pool(name="ps_big", bufs=6, space="PSUM") as ps_big,
            tc.tile_pool(name="ps_small", bufs=2, space="PSUM") as ps_small,
            tc.tile_pool(name="dram", bufs=2, space="DRAM") as dram,
            tc.tile_pool(name="dramg", bufs=4, space="DRAM") as dramg,
        ):
            # ---- resident weights / tables ----
            wq_sb = pw.tile([128, D_CH, O_LOC], BF, tag="wq")
            nc.sync.dma_start(out=wq_sb[:], in_=wq.rearrange("(c p) o -> p c o", p=128))
            wk_sb = pw.tile([128, D_CH, HD], BF, tag="wk")
            nc.sync.dma_start(out=wk_sb[:], in_=wk.rearrange("(c p) o -> p c o", p=128))
            wv_sb = pw.tile([128, D_CH, HD], BF, tag="wv")
            nc.sync.dma_start(out=wv_sb[:], in_=wv.rearrange("(c p) o -> p c o", p=128))
            cos_sb = pw.tile([HD, S], BF, tag="cos")
            nc.sync.dma_start(out=cos_sb[:], in_=cos[:])
            sin_sb = pw.tile([HD, S], BF, tag="sin")
            nc.sync.dma_start(out=sin_sb[:], in_=sin[:])
            ones_sb = pw.tile([128, 1], BF, tag="ones")
            nc.vector.memset(ones_sb[:], 1.0)

            gathered_tiles = []
            for b in range(B):
                qt_b = pqkv.tile([128, H_LOC, S], BF, tag="qt")
                kt_b = pqkv.tile([128, S], BF, tag="kt")
                v_b = pqkv.tile([128, S_CH, HD], BF, tag="v")

                # ---------------- phase 1: QKV projection + RoPE ----------
                for p in range(P_PER_B):
                    t0 = b * S + p * PANEL
                    s0 = p * PANEL
                    halves = []
                    for q in range(2):
                        hq = phst.tile([128, HCH, PANEL], BF, tag="hsT")
                        hsrc = hsT[q * HCH * 128:(q + 1) * HCH * 128, t0:t0 + PANEL]
                        nc.sync.dma_start(
                            out=hq[:], in_=hsrc.rearrange("(c p) t -> p c t", p=128))
                        halves.append(hq)

                    def hs_chunk(c):
                        return halves[c // HCH][:, c % HCH, :]

                    # Q heads + K, transposed layout [d, t], then RoPE
                    for blk in range(H_LOC + 1):
                        ps_t = ps_big.tile([128, PANEL], F32, tag="mm")
                        for c in range(D_CH):
                            lhs = (wq_sb[:, c, blk * HD:(blk + 1) * HD]
                                   if blk < H_LOC else wk_sb[:, c, :])
                            nc.tensor.matmul(ps_t[:], lhs, hs_chunk(c),
                                             start=(c == 0), stop=(c == D_CH - 1))
                        raw = praw.tile([128, PANEL], BF, tag="raw")
                        nc.scalar.copy(out=raw[:], in_=ps_t[:])
                        dst = (qt_b[:, blk, s0:s0 + PANEL] if blk < H_LOC
                               else kt_b[:, s0:s0 + PANEL])
                        cs = cos_sb[:, s0:s0 + PANEL]
                        sn = sin_sb[:, s0:s0 + PANEL]
                        # rotate-half via a partition-swapped copy (DVE ops
                        # need equal base partitions on both inputs)
                        rsw = praw.tile([128, PANEL], BF, tag="rsw")
                        nc.sync.dma_start(out=rsw[0:HALF, :], in_=raw[HALF:HD, :])
                        nc.sync.dma_start(out=rsw[HALF:HD, :], in_=raw[0:HALF, :])
                        tmp = prt.tile([128, PANEL], BF, tag="ropetmp")
                        nc.vector.tensor_mul(tmp[:], raw[:], cs)
                        nc.vector.tensor_mul(rsw[:], rsw[:], sn)
                        nc.vector.tensor_add(dst, tmp[:], rsw[:])

                    # V in token-major layout [t, d]
                    for tt in range(PANEL // 128):
                        ps_v = ps_big.tile([128, PANEL], F32, tag="mm")
                        for c in range(D_CH):
                            nc.tensor.matmul(
                                ps_v[:, 0:HD],
                                hs_chunk(c)[:, tt * 128:(tt + 1) * 128],
                                wv_sb[:, c, :],
                                start=(c == 0), stop=(c == D_CH - 1))
                        nc.vector.tensor_copy(
                            v_b[:, p * (PANEL // 128) + tt, :], ps_v[:, 0:HD])

                # ---------------- phase 2: attention --------------------
                # column sums of V (shared by the head group)
                ps_sv = ps_small.tile([128, PANEL], F32, tag="small")
                for k8 in range(S_CH):
                    nc.tensor.matmul(ps_sv[:, 0:1], v_b[:, k8, :], ones_sb[:],
                                     start=(k8 == 0), stop=(k8 == S_CH - 1))
                sv_sb = psmall.tile([128, 1], F32, tag="sv")
                nc.vector.tensor_copy(sv_sb[:], ps_sv[:, 0:1])

                attn_t = pattn.tile([128, H_LOC, S], BF, tag="attn")
                r_sb = psmall.tile([H_LOC, S], F32, tag="rsb")

                for h in range(H_LOC):
                    for p in range(P_PER_B):
                        q_sl = qt_b[:, h, p * PANEL:(p + 1) * PANEL]
                        ps_r = ps_small.tile([128, PANEL], F32, tag="small")
                        ps_ot = ps_big.tile([128, PANEL], F32, tag="mm")
                        for k8 in range(S_CH):
                            ps_s = ps_big.tile([128, PANEL], F32, tag="mm")
                            nc.tensor.matmul(
                                ps_s[:], kt_b[:, k8 * 128:(k8 + 1) * 128], q_sl,
                                start=True, stop=True)
                            e_t = pe_pool.tile([128, PANEL], F32, tag="E")
                            nc.scalar.activation(out=e_t[:], in_=ps_s[:],
                                                 func=Exp, scale=SCALE)
                            d_c = pd.tile([128, PANEL], BF, tag="D")
                            nc.vector.tensor_scalar_add(
                                out=d_c[:], in0=e_t[:], scalar1=-1.0)
                            # softmax denominator (minus S): rowsum of D
                            nc.tensor.matmul(ps_r[0:1, :], ones_sb[:], d_c[:],
                                             start=(k8 == 0), stop=(k8 == S_CH - 1))
                            # unnormalized attn.T = SV + V.T @ D
                            nc.tensor.matmul(ps_ot[:], v_b[:, k8, :], d_c[:],
                                             start=(k8 == 0), stop=(k8 == S_CH - 1))
                        r_stage = psmall.tile([1, PANEL], F32, tag="rstage")
                        nc.scalar.copy(out=r_stage[:], in_=ps_r[0:1, :])
                        nc.sync.dma_start(
                            out=r_sb[h:h + 1, p * PANEL:(p + 1) * PANEL],
                            in_=r_stage[:])
                        nc.vector.tensor_add(
                            attn_t[:, h, p * PANEL:(p + 1) * PANEL], ps_ot[:],
                            sv_sb[:, 0:1].to_broadcast((128, PANEL)))

                # normalize: recip of (S + rowsum), broadcast over partitions
                nc.vector.tensor_scalar_add(out=r_sb[:], in0=r_sb[:],
                                            scalar1=float(S))
                nc.vector.reciprocal(r_sb[:], r_sb[:])
                r_dram = dram.tile([H_LOC, S], F32, tag="rdram")
                nc.sync.dma_start(out=r_dram[:], in_=r_sb[:])
                for h in range(H_LOC):
                    r_bc = prt.tile([128, S], F32, tag="rbc")
                    nc.sync.dma_start(
                        out=r_bc[:], in_=r_dram[h:h + 1, :].partition_broadcast(128))
                    nc.vector.tensor_mul(attn_t[:, h, :], attn_t[:, h, :], r_bc[:])

                # ---------------- collective: gather heads ---------------
                bounce_b = dram.tile([O_LOC, S], BF, tag="bounce")
                nc.gpsimd.dma_start(
                    out=bounce_b.rearrange("(h p) t -> p h t", p=128),
                    in_=attn_t[:])
                gathered_b = dramg.tile([O_FULL, S], BF, tag="gather",
                                        addr_space="Shared")
                gathered_tiles.append(gathered_b)
                nc.gpsimd.collective_compute(
                    "AllGather", mybir.AluOpType.bypass,
                    replica_groups=[list(range(n_cores))],
                    ins=[bounce_b[:].opt()], outs=[gathered_b[:].opt()])

            # ---------------- phase 3: output projection ------------------
            # wo reuses wq's pool slot, so it loads only after the last
            # phase-1 use of wq; all phase-3 work therefore runs at the end.
            wo_sb = pw.tile([128, O_CH, OUT_SLICE], BF, tag="wq")
            nc.sync.dma_start(
                out=wo_sb[:], in_=wo.rearrange("(c p) o -> p c o", p=128))
            for b in range(B):
                gathered_b = gathered_tiles[b]
                for tt in range(S_CH):
                    at = pat.tile([128, O_CH, 128], BF, tag="at")
                    src = gathered_b[:, tt * 128:(tt + 1) * 128]
                    nc.gpsimd.dma_start(
                        out=at[:], in_=src.rearrange("(c p) t -> p c t", p=128))
                    ps_o = ps_big.tile([128, PANEL], F32, tag="mm")
                    for c in range(O_CH):
                        nc.tensor.matmul(ps_o[:, 0:OUT_SLICE], at[:, c, :],
                                         wo_sb[:, c, :],
                                         start=(c == 0), stop=(c == O_CH - 1))
                    o_sb = pout.tile([128, OUT_SLICE], F32, tag="osb")
                    nc.scalar.copy(out=o_sb[:], in_=ps_o[:, 0:OUT_SLICE])
                    r0 = b * S + tt * 128
                    nc.sync.dma_start(out=out[r0:r0 + 128, :], in_=o_sb[:])

    # shadow serialization with the wait-splitting post-pass
    orig = nc.to_json_bytes
    nc.to_json_bytes = lambda: _fix_bir_waits(orig())
    return nc


# ---------------------------------------------------------------------------
# host-side: shard inputs, run SPMD on 8 cores, reassemble
# ---------------------------------------------------------------------------
def make_in_maps(cfg, hidden_states, cos, sin, Wq, Wk, Wv, Wo):
    n_cores = cfg["n_cores"]
    B, S, D, HD, H_LOC = cfg["B"], cfg["S"], cfg["D"], cfg["HD"], cfg["H_LOC"]
    O_LOC = H_LOC * HD
    HALF = HD // 2
    KV = Wk.shape[0] // HD  # total kv heads == n_cores

    hs2 = np.asarray(hidden_states, dtype=np.float32).reshape(B * S, D)
    hsT = np.ascontiguousarray(hs2.T).astype(ml_dtypes.bfloat16)
    cos_h = np.asarray(cos, np.float32)[0, :, HALF:].T      # [HALF, S]
    sin_h = np.asarray(sin, np.float32)[0, :, HALF:].T
    cos2 = np.ascontiguousarray(
        np.concatenate([cos_h, cos_h], axis=0)).astype(ml_dtypes.bfloat16)
    sin2 = np.ascontiguousarray(
        np.concatenate([-sin_h, sin_h], axis=0)).astype(ml_dtypes.bfloat16)
    Wq = np.asarray(Wq, np.float32)
    Wk = np.asarray(Wk, np.float32)
    Wv = np.asarray(Wv, np.float32)
    Wo = np.asarray(Wo, np.float32)
    assert KV == n_cores, (KV, n_cores)

    in_maps = []
    for c in range(n_cores):
        wq_c = np.ascontiguousarray(Wq[c * O_LOC:(c + 1) * O_LOC, :].T
                                    ).astype(ml_dtypes.bfloat16)
        wk_c = np.ascontiguousarray(Wk[c * HD:(c + 1) * HD, :].T
                                    ).astype(ml_dtypes.bfloat16)
        wv_c = np.ascontiguousarray(Wv[c * HD:(c + 1) * HD, :].T
                                    ).astype(ml_dtypes.bfloat16)
        out_sl = D // n_cores
        wo_c = np.ascontiguousarray(Wo[c * out_sl:(c + 1) * out_sl, :].T
                                    ).astype(ml_dtypes.bfloat16)
        in_maps.append({
            "hsT": hsT, "wq_t": wq_c, "wk_t": wk_c, "wv_t": wv_c,
            "wo_t": wo_c, "cos_t": cos2, "sin_t": sin2,
        })
    return in_maps


def assemble_output(cfg, results):
    B, S, D = cfg["B"], cfg["S"], cfg["D"]
    parts = [results[c]["out"] for c in range(cfg["n_cores"])]
    full = np.concatenate(parts, axis=1)
    return np.ascontiguousarray(full.reshape(B, S, D), dtype=np.float32)


_NC_CACHE = {}


def kernel(hidden_states, cos, sin, Wq, Wk, Wv, Wo):
    from concourse.bass_utils import run_bass_kernel_spmd
    cfg = CFG_FULL
    in_maps = make_in_maps(cfg, hidden_states, cos, sin, Wq, Wk, Wv, Wo)
    key = "full"
    if key not in _NC_CACHE:
        _NC_CACHE[key] = build_nc(cfg)
    nc = _NC_CACHE[key]
    res = run_bass_kernel_spmd(nc, in_maps, list(range(cfg["n_cores"])),
                               trace=False)
    return assemble_output(cfg, res.results)
